# revision 10
# baseline (speedup 1.0000x reference)
"""Trainium2 Bass kernel for nn_PositionalEncoding_61151744360729.

out[b, s, n, :] = x[b, s, n, :] + ||x[b, s+1, n, :] - x[b, s, n, :]||_2
(with distance 0 at s = S-1).

Sharding: data-parallel on batch across 8 NeuronCores (64 batches/core).

Device layout (prepared host-side): fp16, coordinate-plane separated and
node-padded -- xin[b, c, s, n'] with n' in [0, 26) (node 25 is a zero pad
so every frame span is an even element count -> all DVE operands are
4-byte aligned and contiguous, which keeps the fp16 tensor_tensor ops in
2x perf mode). Each batch carries one extra frame (copy of the last), so
the frame-(S-1) distance is exactly 0 with no special-casing.

Per chunk of f frames (partition p = batch*2 + seq-half):
  DVE  shifted subtract over all 3 planes in one op (plane-tail entries
       are garbage and never consumed)
  ACT  square in place
  PE   identity-matmul accumulate folds the 3 squared planes into PSUM
       (512-column segments, each exactly one PSUM bank)
  ACT  sqrt PSUM -> SBUF fp16
  DVE  one broadcast add produces all 3 output planes
  DMA  out (SWDGE) while input loads ride HWDGE on the idle SP engine.

First/last chunks are half-size to shorten pipeline fill/drain. fp16
end-to-end halves HBM traffic vs f32; rel l2 error ~4e-4, well inside
the 2e-2 gate.
"""

import sys
from contextlib import ExitStack

for _p in ("/opt/trn_rl_repo", "/root/.axon_site/_ro/trn_rl_repo"):
    if _p not in sys.path:
        sys.path.insert(0, _p)

import numpy as np

import concourse.bass as bass
import concourse.tile as tile
from concourse import bacc, mybir
from concourse.bass_utils import run_bass_kernel_spmd

B, S, N, C = 512, 1024, 25, 3
W = 26                     # nodes padded to even count (fp16 4B alignment)
NCORES = 8
BC = B // NCORES           # 64 batches per core
H = 2                      # sequence halves -> 128 partitions
SH = S // H                # 512 frames per half
P = H * BC                 # 128 partitions
CHUNKS = [32, 32, 64, 64, 64, 64, 64, 64, 32, 32]   # frames per chunk
assert sum(CHUNKS) == SH
IN_FLAT = BC * C * (S + 1) * W
OUT_FLAT = BC * C * S * W

_cache = {}


def _build():
    f16 = mybir.dt.float16
    f32 = mybir.dt.float32
    Af = mybir.ActivationFunctionType
    nc = bacc.Bacc(
        "TRN2", target_bir_lowering=False, debug=False, num_devices=NCORES
    )
    xin = nc.dram_tensor("xin", [IN_FLAT], f16, kind="ExternalInput")
    ident = nc.dram_tensor("ident", [P * P], f16, kind="ExternalInput")
    yout = nc.dram_tensor("yout", [OUT_FLAT], f16, kind="ExternalOutput")

    offs = [sum(CHUNKS[:i]) for i in range(len(CHUNKS))]
    K = len(CHUNKS)

    with tile.TileContext(nc) as tc, ExitStack() as ctx:
        pident = ctx.enter_context(tc.tile_pool(name="pident", bufs=1))
        ident_t = pident.tile([P, P], f16)
        nc.sync.dma_start(ident_t[:], bass.AP(ident, 0, [[P, P], [1, P]]))

        pin = ctx.enter_context(tc.tile_pool(name="pin", bufs=5))
        pmid = ctx.enter_context(tc.tile_pool(name="pmid", bufs=2))
        psm = ctx.enter_context(tc.tile_pool(name="psm", bufs=3))
        pout = ctx.enter_context(tc.tile_pool(name="pout", bufs=2))
        pps = ctx.enter_context(
            tc.tile_pool(name="pps", bufs=2, space="PSUM")
        )

        PF = 4  # input prefetch depth (chunks)

        def issue_in(k):
            f = CHUNKS[k]
            ispan = (f + 1) * W
            t = pin.tile([P, C * ispan], f16)
            for c in range(C):
                src = bass.AP(
                    xin,
                    c * (S + 1) * W + offs[k] * W,
                    [
                        [C * (S + 1) * W, BC],
                        [SH * W, H],
                        [1, ispan],
                    ],
                )
                nc.sync.dma_start(t[:, c * ispan:(c + 1) * ispan], src)
            return t

        in_tiles = [issue_in(k) for k in range(PF)]

        for k in range(K):
            f = CHUNKS[k]
            ispan = (f + 1) * W          # per-plane input span
            ospan = f * W                # per-plane output span
            subl = C * ispan - W         # one shifted sub across planes
            in_t = in_tiles[k]

            diff_t = pmid.tile([P, C * ispan], f16)
            nc.vector.tensor_sub(
                diff_t[:, 0:subl], in_t[:, W:W + subl], in_t[:, 0:subl]
            )
            nc.scalar.activation(
                diff_t[:, 0:subl], diff_t[:, 0:subl], Af.Square
            )

            # 3-plane fold on PE: identity matmuls accumulating into PSUM.
            nseg = (ospan + 511) // 512
            ps_t = pps.tile([P, nseg * 512], f32)
            for si in range(nseg):
                s0 = si * 512
                sw = min(512, ospan - s0)
                for c in range(C):
                    nc.tensor.matmul(
                        out=ps_t[:, s0:s0 + sw],
                        lhsT=ident_t[:],
                        rhs=diff_t[:, c * ispan + s0:c * ispan + s0 + sw],
                        start=(c == 0),
                        stop=(c == C - 1),
                    )

            dist_t = psm.tile([P, ospan], f16)
            nc.scalar.activation(dist_t[:], ps_t[:, 0:ospan], Af.Sqrt)

            if k + PF < K:
                in_tiles.append(issue_in(k + PF))

            out_t = pout.tile([P, C * ospan], f16)
            out3 = out_t[:].rearrange("p (c l) -> p c l", c=C)
            in3 = in_t[:].rearrange("p (c l) -> p c l", c=C)[:, :, 0:ospan]
            db = dist_t[:].unsqueeze(1).broadcast_to([P, C, ospan])
            nc.vector.tensor_add(out3, in3, db)

            for c in range(C):
                dst = bass.AP(
                    yout,
                    c * S * W + offs[k] * W,
                    [
                        [C * S * W, BC],
                        [SH * W, H],
                        [1, ospan],
                    ],
                )
                nc.gpsimd.dma_start(
                    dst, out_t[:, c * ospan:(c + 1) * ospan]
                )

    nc.compile()
    return nc


def kernel(x: np.ndarray, **_unused) -> np.ndarray:
    x = np.asarray(x)
    assert x.shape == (B, S, N, C), x.shape

    if "nc" not in _cache:
        _cache["nc"] = _build()
    nc = _cache["nc"]

    # [B, S, N, C] f32 -> [B, C, S, N] fp16, node-padded to W, one extra
    # frame per batch (copy of the last -> distance 0 at s = S-1).
    xt = x.astype(np.float16).transpose(0, 3, 1, 2)  # [B, C, S, N]
    ident = np.eye(P, dtype=np.float16).reshape(P * P)
    in_maps = []
    for ci in range(NCORES):
        xp = np.zeros((BC, C, S + 1, W), dtype=np.float16)
        xc = xt[ci * BC:(ci + 1) * BC]
        xp[:, :, :S, :N] = xc
        xp[:, :, S, :N] = xc[:, :, S - 1, :]
        in_maps.append({"xin": xp.reshape(IN_FLAT), "ident": ident})

    res = run_bass_kernel_spmd(nc, in_maps, core_ids=list(range(NCORES)))
    _cache["last_results"] = res

    out = np.empty((B, S, N, C), dtype=np.float32)
    for ci in range(NCORES):
        yo = res.results[ci]["yout"].reshape(BC, C, S, W)
        out[ci * BC:(ci + 1) * BC] = (
            yo[:, :, :, :N].transpose(0, 2, 3, 1).astype(np.float32)
        )
    return out


# revision 16
# speedup vs baseline: 1.2401x; 1.2401x over previous
"""Trainium2 Bass kernel for nn_PositionalEncoding_61151744360729.

out[b, s, n, :] = x[b, s, n, :] + ||x[b, s+1, n, :] - x[b, s, n, :]||_2
(with distance 0 at s = S-1).

Sharding: data-parallel on batch across 8 NeuronCores (64 batches/core).

Device layout (prepared host-side): fp16, coordinate-plane separated and
node-padded -- xin[b, c, s, n'] with n' in [0, 26) (node 25 is a zero pad
so every frame span is an even element count -> all DVE operands are
4-byte aligned and contiguous, which keeps the fp16 tensor_tensor ops in
2x perf mode). Each batch carries one extra frame (copy of the last), so
the frame-(S-1) distance is exactly 0 with no special-casing.

Software-pipelined over chunks with a 3-stage skew so the DVE stream
never waits on ACT inside an iteration:
  iter k issues  DVE sub(k) | ACT square(k) | DVE sums(k-1) |
                 ACT sqrt(k-1) | DVE broadcast-add(k-2) | DMA out(k-2)
Input loads ride HWDGE on the otherwise idle SP engine; output stores
ride SWDGE on GpSimd. fp16 end-to-end halves HBM traffic vs f32; rel l2
error ~4e-4, well inside the 2e-2 gate.
"""

import sys
from contextlib import ExitStack

for _p in ("/opt/trn_rl_repo", "/root/.axon_site/_ro/trn_rl_repo"):
    if _p not in sys.path:
        sys.path.insert(0, _p)

import numpy as np

import concourse.bass as bass
import concourse.tile as tile
from concourse import bacc, mybir
from concourse.bass_utils import run_bass_kernel_spmd

B, S, N, C = 512, 1024, 25, 3
W = 26                     # nodes padded to even count (fp16 4B alignment)
NCORES = 8
BC = B // NCORES           # 64 batches per core
H = 2                      # sequence halves -> 128 partitions
SH = S // H                # 512 frames per half
P = H * BC                 # 128 partitions
CHUNKS = [64, 64, 64, 64, 64, 64, 64, 32, 32]   # frames per chunk
assert sum(CHUNKS) == SH
IN_FLAT = BC * C * (S + 1) * W
OUT_FLAT = BC * C * S * W

_cache = {}


def _build():
    f16 = mybir.dt.float16
    Af = mybir.ActivationFunctionType
    nc = bacc.Bacc(
        "TRN2", target_bir_lowering=False, debug=False, num_devices=NCORES
    )
    xin = nc.dram_tensor("xin", [IN_FLAT], f16, kind="ExternalInput")
    yout = nc.dram_tensor("yout", [OUT_FLAT], f16, kind="ExternalOutput")

    offs = [sum(CHUNKS[:i]) for i in range(len(CHUNKS))]
    K = len(CHUNKS)
    PF = 3  # input prefetch depth (chunks)

    with tile.TileContext(nc) as tc, ExitStack() as ctx:
        pin = ctx.enter_context(tc.tile_pool(name="pin", bufs=PF + 4))
        pmid = ctx.enter_context(tc.tile_pool(name="pmid", bufs=3))
        psm = ctx.enter_context(tc.tile_pool(name="psm", bufs=4))
        pout = ctx.enter_context(tc.tile_pool(name="pout", bufs=2))

        def issue_in(k):
            f = CHUNKS[k]
            ispan = (f + 1) * W
            t = pin.tile([P, C * ispan], f16)
            for c in range(C):
                src = bass.AP(
                    xin,
                    c * (S + 1) * W + offs[k] * W,
                    [
                        [C * (S + 1) * W, BC],
                        [SH * W, H],
                        [1, ispan],
                    ],
                )
                nc.sync.dma_start(t[:, c * ispan:(c + 1) * ispan], src)
            return t

        in_tiles = [issue_in(k) for k in range(PF)]
        diff_tiles = {}
        dist_tiles = {}

        for k in range(K + 2):
            if k < K:
                f = CHUNKS[k]
                ispan = (f + 1) * W
                subl = C * ispan - W
                in_t = in_tiles[k]
                diff_t = pmid.tile([P, C * ispan], f16)
                diff_tiles[k] = diff_t
                if k < 3:
                    # ramp: per-plane so DVE starts as each plane lands
                    for c in range(C):
                        o = c * ispan
                        nc.vector.tensor_sub(
                            diff_t[:, o:o + f * W],
                            in_t[:, o + W:o + W + f * W],
                            in_t[:, o:o + f * W],
                        )
                        nc.scalar.activation(
                            diff_t[:, o:o + f * W],
                            diff_t[:, o:o + f * W],
                            Af.Square,
                        )
                else:
                    nc.vector.tensor_sub(
                        diff_t[:, 0:subl], in_t[:, W:W + subl],
                        in_t[:, 0:subl]
                    )
                    nc.scalar.activation(
                        diff_t[:, 0:subl], diff_t[:, 0:subl], Af.Square
                    )

            j = k - 1
            if 0 <= j < K:
                f = CHUNKS[j]
                ispan = (f + 1) * W
                ospan = f * W
                diff_t = diff_tiles[j]
                dist_t = psm.tile([P, ospan], f16)
                dist_tiles[j] = dist_t
                nc.vector.tensor_add(
                    dist_t[:], diff_t[:, 0:ospan],
                    diff_t[:, ispan:ispan + ospan],
                )
                nc.vector.tensor_add(
                    dist_t[:], dist_t[:],
                    diff_t[:, 2 * ispan:2 * ispan + ospan],
                )
                nc.scalar.activation(dist_t[:], dist_t[:], Af.Sqrt)

            i = k - 2
            if i >= 0:
                f = CHUNKS[i]
                ispan = (f + 1) * W
                ospan = f * W
                in_t = in_tiles[i]
                dist_t = dist_tiles.pop(i)
                in3 = in_t[:].rearrange(
                    "p (c l) -> p c l", c=C, l=ispan
                )[:, :, 0:ospan]
                db = dist_t[:].unsqueeze(1).broadcast_to([P, C, ospan])
                nc.vector.tensor_add(in3, in3, db)
                for c in range(C):
                    dst = bass.AP(
                        yout,
                        c * S * W + offs[i] * W,
                        [
                            [C * S * W, BC],
                            [SH * W, H],
                            [1, ospan],
                        ],
                    )
                    nc.sync.dma_start(
                        dst, in_t[:, c * ispan:c * ispan + ospan]
                    )

            if k + PF < K:
                in_tiles.append(issue_in(k + PF))

    nc.compile()
    return nc


def kernel(x: np.ndarray, **_unused) -> np.ndarray:
    x = np.asarray(x)
    assert x.shape == (B, S, N, C), x.shape

    if "nc" not in _cache:
        _cache["nc"] = _build()
    nc = _cache["nc"]

    # [B, S, N, C] f32 -> [B, C, S, N] fp16, node-padded to W, one extra
    # frame per batch (copy of the last -> distance 0 at s = S-1).
    xt = x.astype(np.float16).transpose(0, 3, 1, 2)  # [B, C, S, N]
    in_maps = []
    for ci in range(NCORES):
        xp = np.zeros((BC, C, S + 1, W), dtype=np.float16)
        xc = xt[ci * BC:(ci + 1) * BC]
        xp[:, :, :S, :N] = xc
        xp[:, :, S, :N] = xc[:, :, S - 1, :]
        in_maps.append({"xin": xp.reshape(IN_FLAT)})

    res = run_bass_kernel_spmd(nc, in_maps, core_ids=list(range(NCORES)))
    _cache["last_results"] = res

    out = np.empty((B, S, N, C), dtype=np.float32)
    for ci in range(NCORES):
        yo = res.results[ci]["yout"].reshape(BC, C, S, W)
        out[ci * BC:(ci + 1) * BC] = (
            yo[:, :, :, :N].transpose(0, 2, 3, 1).astype(np.float32)
        )
    return out
